# revision 13
# baseline (speedup 1.0000x reference)
# Trainium2 Bass kernel for nn_MCorrLCorr (Mellin-correlation along x,
# linear correlation along y).
#
#   out[b,o,hx,hy] = bias[o]
#     + sum_{c,fx,fy} input[b, c, (hx+1)*(fx+1)-1, 2*hy + fy - 2] * weight[o,c,fx,fy]
#   (terms with 2*hy+fy-2 < 0 dropped; only hy=0, fy<2)
#
# Strategy:
#   - data-parallel over batch: 16 batches -> 8 cores, 2 per core.
#   - per core: x-gather is folded into 4 strided DMAs (one per fx) building
#     an SBUF tile X[(fx,c)=128 partitions, hx=32, t=386] where t=2 holds gy=0
#     and t in {0,1} is zero padding (absorbs the dropped y terms).
#   - the y-correlation becomes 8 PSUM-accumulated matmuls per output tile:
#     contraction K=(fx,c)=128, stationary weight[K, o=64], moving operand a
#     stride-2 view X[:, hx0:hx0+2, fy : fy+379 : 2]  (N=2*190=380).
#   - float32r single-pass fp32 matmul mode (full PE rate at N>=256).
#   - bias added on the scalar engine while copying PSUM->SBUF, then DMA out.

import numpy as np

import concourse.bass as bass
import concourse.mybir as mybir
import concourse.tile as tile
from concourse import bacc
from concourse.bass_utils import run_bass_kernel_spmd

B, C, NGX, NGY = 16, 32, 128, 384
O, NFX, NFY = 64, 4, 8
NHX, NHY = 32, 190
NCORES = 8
BPC = B // NCORES  # batches per core
PITCH = NGY + 2  # x-gather row: [0, 0, gy=0 .. gy=383]; t = gy + 2
F32 = mybir.dt.float32
F32R = mybir.dt.float32r

HX_TILE = 2  # output hx rows per PSUM tile (N = HX_TILE*NHY <= 512)


def build_nc():
    nc = bacc.Bacc("TRN2", target_bir_lowering=False)
    inp = nc.dram_tensor("input", [BPC, C, NGX, NGY], F32, kind="ExternalInput")
    wre = nc.dram_tensor("weight", [NFX * C, NFY, O], F32, kind="ExternalInput")
    bia = nc.dram_tensor("bias", [O, 1], F32, kind="ExternalInput")
    out = nc.dram_tensor("out", [BPC, O, NHX, NHY], F32, kind="ExternalOutput")
    inp_ap, wre_ap, bia_ap, out_ap = inp.ap(), wre.ap(), bia.ap(), out.ap()

    with tile.TileContext(nc) as tc:
        with (
            tc.tile_pool(name="consts", bufs=1) as consts,
            tc.tile_pool(name="xp", bufs=2) as xpool,
            tc.tile_pool(name="ob", bufs=4) as opool,
            tc.tile_pool(name="ps", bufs=8, space="PSUM") as pspool,
        ):
            w_sb = consts.tile([NFX * C, NFY, O], F32R)
            nc.sync.dma_start(out=w_sb, in_=wre_ap.bitcast(F32R))
            bias_sb = consts.tile([O, 1], F32)
            nc.sync.dma_start(out=bias_sb, in_=bia_ap)
            zeros_t = nc.inline_tensor(
                np.zeros((NFX * C, NHX, 2), np.float32), name="zeros_pad"
            )
            zeros_ap = zeros_t.ap().bitcast(F32R)

            for b in range(BPC):
                # X[(fx,c), hx, t]: t=0,1 zero; t=2+gy = input[b, c, (hx+1)*(fx+1)-1, gy]
                x_pad = xpool.tile([NFX * C, NHX, PITCH], F32R)
                nc.sync.dma_start(out=x_pad[:, :, 0:2], in_=zeros_ap)
                for fx in range(NFX):
                    src = bass.AP(
                        inp_ap.tensor,
                        b * C * NGX * NGY + fx * NGY,
                        [[NGX * NGY, C], [(fx + 1) * NGY, NHX], [1, NGY]],
                    ).bitcast(F32R)
                    nc.sync.dma_start(
                        out=x_pad[fx * C : (fx + 1) * C, :, 2:PITCH], in_=src
                    )

                for hx0 in range(0, NHX, HX_TILE):
                    ps = pspool.tile([O, HX_TILE, NHY], F32)
                    for fy in range(NFY):
                        # moving operand: t = 2*hy + fy  (hy = 0..189)
                        rhs = x_pad[
                            :, hx0 : hx0 + HX_TILE, fy : fy + 2 * NHY - 1 : 2
                        ]
                        nc.tensor.matmul(
                            ps,
                            w_sb[:, fy, :],
                            rhs,
                            start=(fy == 0),
                            stop=(fy == NFY - 1),
                        )
                    ob = opool.tile([O, HX_TILE, NHY], F32)
                    nc.scalar.add(ob, ps, bias_sb)
                    nc.sync.dma_start(
                        out=out_ap[b, :, hx0 : hx0 + HX_TILE, :], in_=ob
                    )
    nc.compile()
    return nc


def _prep_maps(inputs):
    inp = np.ascontiguousarray(np.asarray(inputs["input"], dtype=np.float32))
    w = np.asarray(inputs["weight"], dtype=np.float32)
    bias = np.asarray(inputs["bias"], dtype=np.float32)
    # W_re[fx*C + c, fy, o] = weight[o, c, fx, fy]
    wre = np.ascontiguousarray(w.transpose(2, 1, 3, 0).reshape(NFX * C, NFY, O))
    bre = np.ascontiguousarray(bias.reshape(O, 1))
    return [
        {
            "input": np.ascontiguousarray(inp[k * BPC : (k + 1) * BPC]),
            "weight": wre,
            "bias": bre,
        }
        for k in range(NCORES)
    ]


def kernel(**inputs) -> np.ndarray:
    nc = build_nc()
    in_maps = _prep_maps(inputs)
    res = run_bass_kernel_spmd(nc, in_maps, core_ids=list(range(NCORES)))
    return np.concatenate([r["out"] for r in res.results], axis=0)


# revision 15
# speedup vs baseline: 1.1332x; 1.1332x over previous
# Trainium2 Bass kernel for nn_MCorrLCorr (Mellin-correlation along x,
# linear correlation along y).
#
#   out[b,o,hx,hy] = bias[o]
#     + sum_{c,fx,fy} input[b, c, (hx+1)*(fx+1)-1, 2*hy + fy - 2] * weight[o,c,fx,fy]
#   (terms with 2*hy+fy-2 < 0 dropped; only hy=0, fy<2)
#
# Strategy:
#   - data-parallel over batch: 16 batches -> 8 cores, 2 per core.
#   - per core: the x-gather is folded into 4 strided DMAs (one per fx)
#     building an SBUF tile X[(fx,c)=128, hx=32, t=388], where t=2+gy and
#     t in {0,1,386,387} is zero padding (absorbs out-of-range y terms).
#   - same-parity fy pairs (fy, fy+2) share one moving stream shifted by one
#     hy: with stationary [W_fy | W_fy+2] (K=128 x M=128, full PE array) a
#     single matmul over X[:, hx, fy : fy+383 : 2] (N=2x192) computes both.
#     PSUM rows 0:64 hold sum_fy_lo at hy=n, rows 64:128 hold sum_fy_hi at
#     hy=n-1; 4 pairs accumulate into one PSUM bank.
#   - weight stationary is swept over 8 PSUM banks back-to-back so the
#     in-array weight load amortizes over 8 matmuls.
#   - float32r single-pass fp32 matmul mode (full PE rate, N>=256, even N).
#   - combine: ACT adds bias while copying rows 0:64; DVE adds the
#     hy-shifted rows 64:128; then DMA out.

import numpy as np

import concourse.bass as bass
import concourse.mybir as mybir
import concourse.tile as tile
from concourse import bacc
from concourse.bass_utils import run_bass_kernel_spmd

B, C, NGX, NGY = 16, 32, 128, 384
O, NFX, NFY = 64, 4, 8
NHX, NHY = 32, 190
NCORES = 8
BPC = B // NCORES  # batches per core
PITCH = NGY + 4  # x row: [0, 0, gy=0..383, 0, 0]; t = gy + 2
F32 = mybir.dt.float32
F32R = mybir.dt.float32r

HX_TILE = 2  # output hx rows per PSUM tile
NMM = NHY + 2  # moving columns per hx row (even, covers hy=0..189 + shift)
PAIR_LO = (0, 1, 4, 5)  # fy pairs (lo, lo+2)
NGRP = 8  # PSUM banks swept per stationary load


def build_nc():
    nc = bacc.Bacc("TRN2", target_bir_lowering=False)
    inp = nc.dram_tensor("input", [BPC, C, NGX, NGY], F32, kind="ExternalInput")
    wre = nc.dram_tensor("weight", [NFX * C, len(PAIR_LO), 128], F32, kind="ExternalInput")
    bia = nc.dram_tensor("bias", [O, 1], F32, kind="ExternalInput")
    out = nc.dram_tensor("out", [BPC, O, NHX, NHY], F32, kind="ExternalOutput")
    inp_ap, wre_ap, bia_ap, out_ap = inp.ap(), wre.ap(), bia.ap(), out.ap()

    with tile.TileContext(nc) as tc:
        with (
            tc.tile_pool(name="consts", bufs=1) as consts,
            tc.tile_pool(name="xp", bufs=2) as xpool,
            tc.tile_pool(name="ob", bufs=4) as opool,
            tc.tile_pool(name="ps", bufs=8, space="PSUM") as pspool,
        ):
            w_sb = consts.tile([NFX * C, len(PAIR_LO), 128], F32R)
            nc.sync.dma_start(out=w_sb, in_=wre_ap.bitcast(F32R))
            bias_sb = consts.tile([O, 1], F32)
            nc.sync.dma_start(out=bias_sb, in_=bia_ap)
            zeros_t = nc.inline_tensor(
                np.zeros((NFX * C, NHX, 2), np.float32), name="zeros_pad"
            )
            zeros_ap = zeros_t.ap().bitcast(F32R)

            for b in range(BPC):
                # X[(fx,c), hx, t]: t=2+gy = input[b, c, (hx+1)*(fx+1)-1, gy]
                x_pad = xpool.tile([NFX * C, NHX, PITCH], F32R)
                nc.sync.dma_start(out=x_pad[:, :, 0:2], in_=zeros_ap)
                nc.sync.dma_start(out=x_pad[:, :, PITCH - 2 : PITCH], in_=zeros_ap)
                for fx in range(NFX):
                    src = bass.AP(
                        inp_ap.tensor,
                        b * C * NGX * NGY + fx * NGY,
                        [[NGX * NGY, C], [(fx + 1) * NGY, NHX], [1, NGY]],
                    ).bitcast(F32R)
                    nc.sync.dma_start(
                        out=x_pad[fx * C : (fx + 1) * C, :, 2 : 2 + NGY], in_=src
                    )

                for half in range(NHX // HX_TILE // NGRP):  # supergroups of 8 banks
                    pss = [
                        pspool.tile(
                            [128, HX_TILE, NMM], F32, tag="ps", name=f"ps_{b}_{half}_{g}"
                        )
                        for g in range(NGRP)
                    ]
                    for pr, fy_lo in enumerate(PAIR_LO):
                        for g in range(NGRP):
                            hx0 = (half * NGRP + g) * HX_TILE
                            # moving: t = fy_lo + 2n, n = 0..NMM-1
                            rhs = x_pad[
                                :, hx0 : hx0 + HX_TILE, fy_lo : fy_lo + 2 * NMM - 1 : 2
                            ]
                            nc.tensor.matmul(
                                pss[g],
                                w_sb[:, pr, :],
                                rhs,
                                start=(pr == 0),
                                stop=(pr == len(PAIR_LO) - 1),
                            )
                    for g in range(NGRP):
                        hx0 = (half * NGRP + g) * HX_TILE
                        ps = pss[g]
                        ob = opool.tile([O, HX_TILE, NHY], F32)
                        # rows 0:64: fy_lo sums at hy=n; add bias while copying
                        nc.scalar.add(ob, ps[0:64, :, 0:NHY], bias_sb)
                        # rows 64:128: fy_hi sums at hy=n-1 -> shift left by one
                        nc.vector.tensor_add(
                            ob, ob, ps[64:128, :, 1 : NHY + 1]
                        )
                        nc.sync.dma_start(
                            out=out_ap[b, :, hx0 : hx0 + HX_TILE, :], in_=ob
                        )
    nc.compile()
    return nc


def _prep_maps(inputs):
    inp = np.ascontiguousarray(np.asarray(inputs["input"], dtype=np.float32))
    w = np.asarray(inputs["weight"], dtype=np.float32)
    bias = np.asarray(inputs["bias"], dtype=np.float32)
    # wt[fx*C + c, fy, o] = weight[o, c, fx, fy]
    wt = w.transpose(2, 1, 3, 0).reshape(NFX * C, NFY, O)
    w2 = np.empty((NFX * C, len(PAIR_LO), 128), np.float32)
    for pr, fy_lo in enumerate(PAIR_LO):
        w2[:, pr, 0:O] = wt[:, fy_lo]
        w2[:, pr, O:128] = wt[:, fy_lo + 2]
    w2 = np.ascontiguousarray(w2)
    bre = np.ascontiguousarray(bias.reshape(O, 1))
    return [
        {
            "input": np.ascontiguousarray(inp[k * BPC : (k + 1) * BPC]),
            "weight": w2,
            "bias": bre,
        }
        for k in range(NCORES)
    ]


def kernel(**inputs) -> np.ndarray:
    nc = build_nc()
    in_maps = _prep_maps(inputs)
    res = run_bass_kernel_spmd(nc, in_maps, core_ids=list(range(NCORES)))
    return np.concatenate([r["out"] for r in res.results], axis=0)


# revision 16
# speedup vs baseline: 1.3371x; 1.1799x over previous
# Trainium2 Bass kernel for nn_MCorrLCorr (Mellin-correlation along x,
# linear correlation along y).
#
#   out[b,o,hx,hy] = bias[o]
#     + sum_{c,fx,fy} input[b, c, (hx+1)*(fx+1)-1, 2*hy + fy - 2] * weight[o,c,fx,fy]
#   (terms with 2*hy+fy-2 < 0 dropped; only hy=0, fy<2)
#
# Strategy:
#   - data-parallel over batch: 16 batches -> 8 cores, 2 per core.
#   - the x-gather is folded into strided SWDGE DMAs (one per fx per
#     16-hx chunk) that also CAST fp32 -> bf16 in flight, building SBUF
#     tiles X[(fx,c)=128, hx=16, t=388], t=2+gy with zero padding at
#     t in {0,1,386,387} (absorbs out-of-range y terms). Chunking by hx
#     half lets the PE start while the rest of the input streams in.
#   - same-parity fy pairs (fy, fy+2) share one moving stream shifted by
#     one hy: with stationary [W_fy | W_fy+2] (K=128 x M=128) a single
#     matmul over X[:, hx, fy : fy+383 : 2] (N=2x192) computes both fy.
#     PSUM rows 0:64 hold fy_lo sums at hy=n, rows 64:128 hold fy_hi at
#     hy=n-1; the 4 pairs accumulate into one PSUM bank.
#   - each stationary is swept over 8 PSUM banks back-to-back to amortize
#     weight loads; bf16 gets fast-weight-load automatically.
#   - combine: ACT adds bias while copying rows 0:64; DVE adds the
#     hy-shifted rows 64:128; DMA out.

import ml_dtypes
import numpy as np

import concourse.bass as bass
import concourse.mybir as mybir
import concourse.tile as tile
from concourse import bacc
from concourse.bass_utils import run_bass_kernel_spmd

B, C, NGX, NGY = 16, 32, 128, 384
O, NFX, NFY = 64, 4, 8
NHX, NHY = 32, 190
NCORES = 8
BPC = B // NCORES  # batches per core
PITCH = NGY + 4  # x row: [0, 0, gy=0..383, 0, 0]; t = gy + 2
F32 = mybir.dt.float32
BF16 = mybir.dt.bfloat16

HX_TILE = 2  # output hx rows per PSUM tile
NMM = NHY + 2  # moving columns per hx row (covers hy=0..189 + shift)
PAIR_LO = (0, 1, 4, 5)  # fy pairs (lo, lo+2)
NGRP = 8  # PSUM banks swept per stationary load
HCH = NGRP * HX_TILE  # hx rows per chunk (16)
NCHUNK = NHX // HCH  # chunks per batch (2)


def build_nc():
    nc = bacc.Bacc("TRN2", target_bir_lowering=False)
    inp = nc.dram_tensor("input", [BPC, C, NGX, NGY], F32, kind="ExternalInput")
    wre = nc.dram_tensor(
        "weight", [NFX * C, len(PAIR_LO), 128], BF16, kind="ExternalInput"
    )
    bia = nc.dram_tensor("bias", [O, 1], F32, kind="ExternalInput")
    out = nc.dram_tensor("out", [BPC, O, NHX, NHY], F32, kind="ExternalOutput")
    inp_ap, wre_ap, bia_ap, out_ap = inp.ap(), wre.ap(), bia.ap(), out.ap()

    with tile.TileContext(nc) as tc:
        with (
            tc.tile_pool(name="consts", bufs=1) as consts,
            tc.tile_pool(name="xp", bufs=3) as xpool,
            tc.tile_pool(name="ob", bufs=4) as opool,
            tc.tile_pool(name="ps", bufs=8, space="PSUM") as pspool,
        ):
            w_sb = consts.tile([NFX * C, len(PAIR_LO), 128], BF16)
            nc.sync.dma_start(out=w_sb, in_=wre_ap)
            bias_sb = consts.tile([O, 1], F32)
            nc.sync.dma_start(out=bias_sb, in_=bia_ap)
            zeros_t = nc.inline_tensor(
                np.zeros((NFX * C, HCH, 2), ml_dtypes.bfloat16), name="zeros_pad"
            )
            zeros_ap = zeros_t.ap()

            for b in range(BPC):
                for ch in range(NCHUNK):
                    hxb = ch * HCH  # first global hx row of this chunk
                    # X[(fx,c), l, t]: t=2+gy = input[b, c, (hxb+l+1)*(fx+1)-1, gy]
                    xh = xpool.tile(
                        [NFX * C, HCH, PITCH], BF16, tag="xh", name=f"xh_{b}_{ch}"
                    )
                    nc.sync.dma_start(out=xh[:, :, 0:2], in_=zeros_ap)
                    nc.sync.dma_start(out=xh[:, :, PITCH - 2 : PITCH], in_=zeros_ap)
                    for fx in range(NFX):
                        row0 = (hxb + 1) * (fx + 1) - 1
                        src = bass.AP(
                            inp_ap.tensor,
                            b * C * NGX * NGY + row0 * NGY,
                            [[NGX * NGY, C], [(fx + 1) * NGY, HCH], [1, NGY]],
                        )
                        # SWDGE DMA casts fp32 -> bf16 in flight
                        nc.gpsimd.dma_start(
                            out=xh[fx * C : (fx + 1) * C, :, 2 : 2 + NGY], in_=src
                        )

                    pss = [
                        pspool.tile(
                            [128, HX_TILE, NMM], F32, tag="ps", name=f"ps_{b}_{ch}_{g}"
                        )
                        for g in range(NGRP)
                    ]
                    for pr in range(len(PAIR_LO)):
                        fy_lo = PAIR_LO[pr]
                        for g in range(NGRP):
                            l0 = g * HX_TILE
                            # moving: t = fy_lo + 2n, n = 0..NMM-1
                            rhs = xh[
                                :, l0 : l0 + HX_TILE, fy_lo : fy_lo + 2 * NMM - 1 : 2
                            ]
                            nc.tensor.matmul(
                                pss[g],
                                w_sb[:, pr, :],
                                rhs,
                                start=(pr == 0),
                                stop=(pr == len(PAIR_LO) - 1),
                            )
                    for g in range(NGRP):
                        hx0 = hxb + g * HX_TILE
                        ps = pss[g]
                        ob = opool.tile(
                            [O, HX_TILE, NHY], F32, tag="ob", name=f"ob_{b}_{ch}_{g}"
                        )
                        # rows 0:64: fy_lo sums at hy=n; add bias while copying
                        nc.scalar.add(ob, ps[0:64, :, 0:NHY], bias_sb)
                        # rows 64:128: fy_hi sums at hy=n-1 -> shift left by one
                        nc.vector.tensor_add(ob, ob, ps[64:128, :, 1 : NHY + 1])
                        nc.sync.dma_start(
                            out=out_ap[b, :, hx0 : hx0 + HX_TILE, :], in_=ob
                        )
    nc.compile()
    return nc


def _prep_maps(inputs):
    inp = np.ascontiguousarray(np.asarray(inputs["input"], dtype=np.float32))
    w = np.asarray(inputs["weight"], dtype=np.float32)
    bias = np.asarray(inputs["bias"], dtype=np.float32)
    # wt[fx*C + c, fy, o] = weight[o, c, fx, fy]
    wt = w.transpose(2, 1, 3, 0).reshape(NFX * C, NFY, O)
    w2 = np.empty((NFX * C, len(PAIR_LO), 128), np.float32)
    for pr, fy_lo in enumerate(PAIR_LO):
        w2[:, pr, 0:O] = wt[:, fy_lo]
        w2[:, pr, O:128] = wt[:, fy_lo + 2]
    w2 = np.ascontiguousarray(w2.astype(ml_dtypes.bfloat16))
    bre = np.ascontiguousarray(bias.reshape(O, 1))
    return [
        {
            "input": np.ascontiguousarray(inp[k * BPC : (k + 1) * BPC]),
            "weight": w2,
            "bias": bre,
        }
        for k in range(NCORES)
    ]


def kernel(**inputs) -> np.ndarray:
    nc = build_nc()
    in_maps = _prep_maps(inputs)
    res = run_bass_kernel_spmd(nc, in_maps, core_ids=list(range(NCORES)))
    return np.concatenate([r["out"] for r in res.results], axis=0)


# revision 17
# speedup vs baseline: 1.3563x; 1.0144x over previous
# Trainium2 Bass kernel for nn_MCorrLCorr (Mellin-correlation along x,
# linear correlation along y).
#
#   out[b,o,hx,hy] = bias[o]
#     + sum_{c,fx,fy} input[b, c, (hx+1)*(fx+1)-1, 2*hy + fy - 2] * weight[o,c,fx,fy]
#   (terms with 2*hy+fy-2 < 0 dropped; only hy=0, fy<2)
#
# Per core (2 batches, data-parallel over 8 cores), pipelined in 16-hx chunks:
#   1. x-gather: 4 strided DMAs per chunk (one per fx) load
#      S[(fx,c)=128, l=16, gy=384] fp32 from HBM, split across the HWDGE
#      (sync) and SWDGE (gpsimd) rings so two transfers run concurrently.
#   2. cast + parity split: DVE copies even gy, ACT copies odd gy, casting
#      fp32 -> bf16 into Xe/Xo[(fx,c), l, 194] so every matmul's moving
#      operand is CONTIGUOUS bf16 (full PE streaming rate). Index 0 / 193
#      are zeros (absorb the dropped out-of-range y terms).
#   3. matmul: same-parity fy pairs (fy, fy+2) share one moving stream
#      shifted by one hy. Stationary [W_fy | W_fy+2] (K=128 x M=128), one
#      matmul over X?[:, l0:l0+2, off:off+192] (N=384) computes both fy:
#      PSUM rows 0:64 = fy_lo sums at hy=n, rows 64:128 = fy_hi at hy=n-1.
#      4 pairs accumulate into one PSUM bank; each stationary sweeps 8
#      PSUM banks back-to-back to amortize the weight load (bf16 gets FWL).
#   4. combine into a per-chunk staging tile: ACT adds bias while copying
#      rows 0:64, DVE adds the hy-shifted rows 64:128; ONE output DMA per
#      chunk (64 x 12 KB contiguous descriptors).

import ml_dtypes
import numpy as np

import concourse.bass as bass
import concourse.mybir as mybir
import concourse.tile as tile
from concourse import bacc
from concourse.bass_utils import run_bass_kernel_spmd

B, C, NGX, NGY = 16, 32, 128, 384
O, NFX, NFY = 64, 4, 8
NHX, NHY = 32, 190
NCORES = 8
BPC = B // NCORES  # batches per core
F32 = mybir.dt.float32
BF16 = mybir.dt.bfloat16

HX_TILE = 2  # output hx rows per PSUM tile
NMM = NHY + 2  # moving columns per matmul per hx row
NPAR = NHY + 4  # parity-tile columns: [zero, 96 gy values..., zero]
PAIR_LO = (0, 1, 4, 5)  # fy pairs (lo, lo+2)
NGRP = 8  # PSUM banks swept per stationary load
HCH = NGRP * HX_TILE  # hx rows per chunk (16)
NCHUNK = NHX // HCH  # chunks per batch (2)


def build_nc():
    nc = bacc.Bacc("TRN2", target_bir_lowering=False)
    inp = nc.dram_tensor("input", [BPC, C, NGX, NGY], F32, kind="ExternalInput")
    wre = nc.dram_tensor(
        "weight", [NFX * C, len(PAIR_LO), 128], BF16, kind="ExternalInput"
    )
    bia = nc.dram_tensor("bias", [O, 1], F32, kind="ExternalInput")
    out = nc.dram_tensor("out", [BPC, O, NHX, NHY], F32, kind="ExternalOutput")
    inp_ap, wre_ap, bia_ap, out_ap = inp.ap(), wre.ap(), bia.ap(), out.ap()

    with tile.TileContext(nc) as tc:
        with (
            tc.tile_pool(name="consts", bufs=1) as consts,
            tc.tile_pool(name="xst", bufs=2) as stpool,
            tc.tile_pool(name="xpar", bufs=2) as parpool,
            tc.tile_pool(name="obc", bufs=2) as opool,
            tc.tile_pool(name="ps", bufs=8, space="PSUM") as pspool,
        ):
            w_sb = consts.tile([NFX * C, len(PAIR_LO), 128], BF16)
            nc.sync.dma_start(out=w_sb, in_=wre_ap)
            bias_sb = consts.tile([O, 1], F32)
            nc.sync.dma_start(out=bias_sb, in_=bia_ap)

            for b in range(BPC):
                for ch in range(NCHUNK):
                    hxb = ch * HCH  # first global hx row of this chunk
                    # S[(fx,c), l, gy] = input[b, c, (hxb+l+1)*(fx+1)-1, gy]
                    xst = stpool.tile(
                        [NFX * C, HCH, NGY], F32, tag="xst", name=f"xst_{b}_{ch}"
                    )
                    for fx in range(NFX):
                        row0 = (hxb + 1) * (fx + 1) - 1
                        src = bass.AP(
                            inp_ap.tensor,
                            b * C * NGX * NGY + row0 * NGY,
                            [[NGX * NGY, C], [(fx + 1) * NGY, HCH], [1, NGY]],
                        )
                        dst = xst[fx * C : (fx + 1) * C, :, :]
                        # split issue across two DMA rings: fx 0,3 -> HWDGE
                        # (sync), fx 1,2 -> SWDGE (gpsimd), cost-balanced by
                        # HBM stride penalty (fx+1).
                        if fx in (0, 3):
                            nc.sync.dma_start(out=dst, in_=src)
                        else:
                            nc.gpsimd.dma_start(out=dst, in_=src)

                    # parity split + cast: X[q][p, l, 1+k] = S[p, l, 2k+q]
                    xe = parpool.tile(
                        [NFX * C, HCH, NPAR], BF16, tag="xe", name=f"xe_{b}_{ch}"
                    )
                    xo = parpool.tile(
                        [NFX * C, HCH, NPAR], BF16, tag="xo", name=f"xo_{b}_{ch}"
                    )
                    nc.gpsimd.memset(xe[:, :, 0:1], 0.0)
                    nc.gpsimd.memset(xe[:, :, NPAR - 1 : NPAR], 0.0)
                    nc.gpsimd.memset(xo[:, :, 0:1], 0.0)
                    nc.gpsimd.memset(xo[:, :, NPAR - 1 : NPAR], 0.0)
                    nc.vector.tensor_copy(xe[:, :, 1 : NPAR - 1], xst[:, :, 0:NGY:2])
                    nc.scalar.copy(xo[:, :, 1 : NPAR - 1], xst[:, :, 1:NGY:2])
                    xq = (xe, xo)

                    pss = [
                        pspool.tile(
                            [128, HX_TILE, NMM], F32, tag="ps", name=f"ps_{b}_{ch}_{g}"
                        )
                        for g in range(NGRP)
                    ]
                    for pr in range(len(PAIR_LO)):
                        fy_lo = PAIR_LO[pr]
                        q, off = fy_lo & 1, (fy_lo - (fy_lo & 1)) // 2
                        for g in range(NGRP):
                            l0 = g * HX_TILE
                            rhs = xq[q][:, l0 : l0 + HX_TILE, off : off + NMM]
                            nc.tensor.matmul(
                                pss[g],
                                w_sb[:, pr, :],
                                rhs,
                                start=(pr == 0),
                                stop=(pr == len(PAIR_LO) - 1),
                            )

                    obc = opool.tile(
                        [O, HCH, NHY], F32, tag="obc", name=f"obc_{b}_{ch}"
                    )
                    for g in range(NGRP):
                        l0 = g * HX_TILE
                        ps = pss[g]
                        ob = obc[:, l0 : l0 + HX_TILE, :]
                        # rows 0:64: fy_lo sums at hy=n; add bias while copying
                        nc.scalar.add(ob, ps[0:64, :, 0:NHY], bias_sb)
                        # rows 64:128: fy_hi sums at hy=n-1 -> shift left by one
                        nc.vector.tensor_add(ob, ob, ps[64:128, :, 1 : NHY + 1])
                    nc.sync.dma_start(
                        out=out_ap[b, :, hxb : hxb + HCH, :], in_=obc
                    )
    nc.compile()
    return nc


def _prep_maps(inputs):
    inp = np.ascontiguousarray(np.asarray(inputs["input"], dtype=np.float32))
    w = np.asarray(inputs["weight"], dtype=np.float32)
    bias = np.asarray(inputs["bias"], dtype=np.float32)
    # wt[fx*C + c, fy, o] = weight[o, c, fx, fy]
    wt = w.transpose(2, 1, 3, 0).reshape(NFX * C, NFY, O)
    w2 = np.empty((NFX * C, len(PAIR_LO), 128), np.float32)
    for pr, fy_lo in enumerate(PAIR_LO):
        w2[:, pr, 0:O] = wt[:, fy_lo]
        w2[:, pr, O:128] = wt[:, fy_lo + 2]
    w2 = np.ascontiguousarray(w2.astype(ml_dtypes.bfloat16))
    bre = np.ascontiguousarray(bias.reshape(O, 1))
    return [
        {
            "input": np.ascontiguousarray(inp[k * BPC : (k + 1) * BPC]),
            "weight": w2,
            "bias": bre,
        }
        for k in range(NCORES)
    ]


def kernel(**inputs) -> np.ndarray:
    nc = build_nc()
    in_maps = _prep_maps(inputs)
    res = run_bass_kernel_spmd(nc, in_maps, core_ids=list(range(NCORES)))
    return np.concatenate([r["out"] for r in res.results], axis=0)
